# revision 3
# baseline (speedup 1.0000x reference)
"""Multi-head self-attention (B=2, S=2048, D=1024, H=16) on 8 TRN2 NeuronCores.

Sharding: data-parallel over batch (2) x tensor-parallel over head-groups (4).
Core c = b*4 + hg handles batch b, heads hg*4..hg*4+3 (4 heads, 256 features).

Per-core device program (SPMD, identical on all cores):
  - QKV projections for the core's 256 output features (column-parallel)
  - full S x S attention for its 4 heads (softmax without max-subtraction,
    denominators via an appended ones-column in the PV matmul)
  - partial output projection (row-parallel): out_partial^T [1024, 2048]
Host: shards/transposes inputs, sums the 4 partial outputs per batch
(the "all-reduce"), adds bo, and untransposes.

Performance structure (v2):
  - The Scalar engine's exp() is the core bottleneck (16.8M elements/core at
    1 elem/cycle/lane). A fraction of exp tiles is offloaded to the Vector
    engine via a custom 8-stage DVE op evaluating a degree-4 polynomial
    (max rel err ~5e-3 before softmax normalization cancels most of it).
  - Inputs stream s-block-major so the first scores/exp start ~10us in
    instead of ~50us; projection matmuls are interleaved chunk-by-chunk into
    the attention loop so the PE fills exp-paced gaps without stalling ACT.
  - 8 junk matmuls at t=0 warm the PE HAM clock gate during the DMA prefix.
"""

import numpy as np

B, S, D = 2, 2048, 1024
H, DK = 16, 64
NCORES = 8
HG = 4          # head groups (tensor parallel)
HPG = 4         # heads per group
F = HPG * DK    # 256 local features per core
SCALE = 1.0 / np.sqrt(DK)

# exp(s/8) ~= 1 + s(c1 + s(c2 + s(c3 + s*c4))), fit on |s/8| <= 1.5,
# positive on all of R; feeds softmax so the ~5e-3 poly error mostly cancels.
PC1 = 0.12456975147852978
PC2 = 0.007996476978709425
PC3 = 0.0003607961408373304
PC4 = 9.209647301994487e-06

_EXP_OP_NAME = "EXP_POLY4_ANT"

_compiled = {}


def _register_exp_op():
    import concourse.dve_ops as dve_ops
    from concourse.dve_spec import (
        Spec, Src0, Src1, C0, C1, C2, One, Bin, AluOp, lower as dve_lower,
        _has_src1,
    )
    from concourse.dve_uop import DveOpSpec

    if _EXP_OP_NAME in dve_ops._SUB_OPCODE_FOR_NAME:
        return next(o for o in dve_ops.OPS if o.name == _EXP_OP_NAME)

    # Horner from the inside: C0=c4, C1=c3, C2=c2, Src1=c1 (broadcast [P,1])
    h = Bin(AluOp.MULTIPLY, Src0, C0) + C1
    h = Bin(AluOp.MULTIPLY, h, Src0) + C2
    h = Bin(AluOp.MULTIPLY, h, Src0) + Src1
    body = Bin(AluOp.MULTIPLY, h, Src0) + One

    def ref(in0, in1, s0, s1, imm2):
        x = in0.astype(np.float32)
        return ((((x * s0 + s1) * x + imm2) * x + in1) * x) + 1.0

    spec = Spec(body=body, reference=ref)
    row = dve_ops._CUSTOM_DVE_ROW_BASE + len(dve_ops.OPS)
    dve_ops._SUB_OPCODE_FOR_NAME[_EXP_OP_NAME] = row
    shas = {}
    for ver in ("v3", "v4"):
        uops = dve_lower(spec, ver=ver)
        shas[ver] = DveOpSpec(
            name=_EXP_OP_NAME, opcode=row, uops=uops, rd1_en=_has_src1(spec)
        ).sha(ver)
    op = dve_ops.DveOp(_EXP_OP_NAME, spec, subdim=False, uops_sha=shas)
    dve_ops.OPS.append(op)
    dve_ops.CUSTOM_DVE_SPECS[_EXP_OP_NAME] = op.spec
    return op


import os
_OFFLOAD_ON = os.environ.get("KERNEL_EXP_OFFLOAD", "1") == "1"


def _offload(seq_idx, jc, jj):
    """Which exp units go to the Vector engine instead of ScalarE."""
    return _OFFLOAD_ON and seq_idx >= 1 and jj == 1 and jc in (1, 3, 5)


def _build():
    import concourse.bacc as bacc
    import concourse.tile as tile
    from concourse import mybir

    exp_op = _register_exp_op()

    f32 = mybir.dt.float32
    bf16 = mybir.dt.bfloat16
    Exp = mybir.ActivationFunctionType.Exp
    mult = mybir.AluOpType.mult

    nc = bacc.Bacc("TRN2", target_bir_lowering=False, debug=False,
                   enable_asserts=True, num_devices=NCORES)

    xq = nc.dram_tensor("xq", (D, S), bf16, kind="ExternalInput")   # q[b].T
    xk = nc.dram_tensor("xk", (D, S), bf16, kind="ExternalInput")
    xv = nc.dram_tensor("xv", (D, S), bf16, kind="ExternalInput")
    wq = nc.dram_tensor("wq", (D, F), bf16, kind="ExternalInput")   # Wq[rows].T
    wk = nc.dram_tensor("wk", (D, F), bf16, kind="ExternalInput")
    wv = nc.dram_tensor("wv", (D, F), bf16, kind="ExternalInput")
    wo = nc.dram_tensor("wo", (F, D), bf16, kind="ExternalInput")   # Wo[:, cols].T
    bq = nc.dram_tensor("bq", (128, 2), f32, kind="ExternalInput")  # bias, f-tiled
    bk = nc.dram_tensor("bk", (128, 2), f32, kind="ExternalInput")
    bv = nc.dram_tensor("bv", (1, F), f32, kind="ExternalInput")
    out = nc.dram_tensor("out", (D, S), bf16, kind="ExternalOutput")  # partial^T

    NDT = D // 128   # 8 d-tiles
    NST = S // 128   # 16 s-tiles (j tiles)
    NSB = S // 512   # 4 s-blocks (i blocks)

    with tile.TileContext(nc) as tc:
        import contextlib
        with contextlib.ExitStack() as ctx:
            consts = ctx.enter_context(tc.tile_pool(name="consts", bufs=1))
            big = ctx.enter_context(tc.tile_pool(name="big", bufs=25))
            atp = ctx.enter_context(tc.tile_pool(name="atp", bufs=9))
            acts = ctx.enter_context(tc.tile_pool(name="acts", bufs=1))
            ostage = ctx.enter_context(tc.tile_pool(name="ostage", bufs=3))
            small = ctx.enter_context(tc.tile_pool(name="small", bufs=2))
            ps = ctx.enter_context(tc.tile_pool(name="ps", bufs=1, space="PSUM"))

            from concourse.tile_rust import add_dep_helper

            # ---- SBUF constants / persistent tiles ----
            wq_sb = consts.tile([128, NDT, F], bf16, tag="wq")
            wk_sb = consts.tile([128, NDT, F], bf16, tag="wk")
            wv_sb = consts.tile([128, NDT, F], bf16, tag="wv")
            wo_sb = consts.tile([128, 2, D], bf16, tag="wo")
            bq_sb = consts.tile([128, 2], f32, tag="bq")
            bk_sb = consts.tile([128, 2], f32, tag="bk")
            bv_sb = consts.tile([128, F], f32, tag="bv")
            c1_sb = consts.tile([128, 1], f32, tag="c1t")   # poly c1 for DVE exp
            junk = consts.tile([128, 512], bf16, tag="junk")
            nc.vector.memset(c1_sb[:], PC1)
            nc.vector.memset(junk[:], 0.125)

            # persistent activations
            qh_t = [[acts.tile([128, 512], bf16, tag=f"qh{ft}{sb}", name=f"qh{ft}{sb}")
                     for sb in range(NSB)] for ft in range(2)]
            kh_t = [[acts.tile([128, 512], bf16, tag=f"kh{ft}{sb}", name=f"kh{ft}{sb}")
                     for sb in range(NSB)] for ft in range(2)]
            vh_t = [acts.tile([128, HPG, DK + 1], bf16, tag=f"vh{st}", name=f"vh{st}")
                    for st in range(NST)]
            for st in range(NST):
                nc.vector.memset(vh_t[st][:, :, DK:DK + 1], 1.0)
            y_t = [[acts.tile([128, 512], bf16, tag=f"y{ft}{sb}", name=f"y{ft}{sb}")
                    for sb in range(NSB)] for ft in range(2)]

            # ---- HAM warmup: keep the PE busy during the DMA prefix so the
            # clock gate is at 8/8 when real matmuls arrive ----
            wps = ps.tile([128, 512], f32, tag="w1", bufs=4, name="warm")
            for i in range(8):
                nc.tensor.matmul(wps[:], junk[:, 0:128], junk[:],
                                 start=(i == 0), stop=(i == 7))
            nc.vector.tensor_copy(junk[:, 0:64], wps[:, 0:64])  # keep it live

            # ---- input DMAs, consumption order, chained so the queue
            # services them in exactly this order ----
            xqt = [big.tile([128, S], bf16, tag="big", name=f"xq{dt}") for dt in range(NDT)]
            xkt = [big.tile([128, S], bf16, tag="big", name=f"xk{dt}") for dt in range(NDT)]
            xvt = [big.tile([128, S], bf16, tag="big", name=f"xv{dt}") for dt in range(NDT)]

            chain = {"prev": None}

            def dma(dst, src):
                d = nc.sync.dma_start(dst, src)
                if chain["prev"] is not None:
                    add_dep_helper(d.ins, chain["prev"].ins, sync=True,
                                   reason="input DMA priority ordering")
                chain["prev"] = d
                return d

            def load_w(w_sb_, wdram):
                for dt in range(NDT):
                    dma(w_sb_[:, dt, :], wdram.ap()[dt * 128:(dt + 1) * 128, :])

            def load_x_sb(ts, xdram, sb):
                for dt in range(NDT):
                    dma(ts[dt][:, sb * 512:(sb + 1) * 512],
                        xdram.ap()[dt * 128:(dt + 1) * 128, sb * 512:(sb + 1) * 512])

            dma(bq_sb[:], bq.ap()[:])
            dma(bk_sb[:], bk.ap()[:])
            dma(bv_sb[:], bv.ap().to_broadcast((128, F)))
            load_w(wk_sb, wk)
            load_x_sb(xkt, xk, 0)
            load_w(wq_sb, wq)
            load_x_sb(xqt, xq, 0)
            load_x_sb(xkt, xk, 1)
            load_x_sb(xkt, xk, 2)
            load_x_sb(xkt, xk, 3)
            load_x_sb(xqt, xq, 1)
            load_w(wv_sb, wv)
            load_x_sb(xvt, xv, 0)
            load_x_sb(xvt, xv, 1)
            load_x_sb(xqt, xq, 2)
            load_x_sb(xvt, xv, 2)
            load_x_sb(xvt, xv, 3)
            load_x_sb(xqt, xq, 3)
            for ft in range(2):
                dma(wo_sb[:, ft, :], wo.ap()[ft * 128:(ft + 1) * 128, :])

            # ---- projection helpers ----
            def qk_sb(w_sb_, b_sb_, xts, dst, ft, sb):
                acc = ps.tile([128, 512], f32, tag="w1", bufs=4,
                              name=f"qk{ft}{sb}")
                for dt in range(NDT):
                    nc.tensor.matmul(
                        acc[:],
                        w_sb_[:, dt, ft * 128:(ft + 1) * 128],
                        xts[dt][:, sb * 512:(sb + 1) * 512],
                        start=(dt == 0), stop=(dt == NDT - 1),
                    )
                nc.vector.tensor_scalar_add(dst[ft][sb][:], acc[:],
                                            b_sb_[:, ft:ft + 1])

            def v_g(g):
                accs = [ps.tile([128, 512], f32, tag="w1", bufs=4,
                                name=f"vps{g}{j}") for j in range(2)]
                for dt in range(NDT):
                    for j in range(2):
                        st = 2 * g + j
                        nc.tensor.matmul(
                            accs[j][:, 0:F],
                            xvt[dt][:, st * 128:(st + 1) * 128],
                            wv_sb[:, dt, :],
                            start=(dt == 0), stop=(dt == NDT - 1),
                        )
                for j in range(2):
                    st = 2 * g + j
                    nc.vector.tensor_tensor(
                        vh_t[st][:, :, 0:DK],
                        accs[j][:, 0:F].rearrange("p (h c) -> p h c", h=HPG),
                        bv_sb[:].rearrange("p (h c) -> p h c", h=HPG),
                        mybir.AluOpType.add,
                    )

            # ---- attention chunk helpers ----
            def scores_chunk(pr, ib, jc, seq_idx):
                ft = pr
                at = atp.tile([128, 4, 512], bf16, tag="at", name=f"at{pr}{ib}{jc}")
                for jj in range(2):
                    jt = jc * 2 + jj
                    sc = ps.tile([128, 2, 512], f32, tag="w2", bufs=2, name="sc")
                    for hh in range(2):
                        base = hh * 64
                        nc.tensor.matmul(
                            sc[:, hh, :],
                            kh_t[ft][jt // 4][base:base + 64,
                                              (jt % 4) * 128:(jt % 4 + 1) * 128],
                            qh_t[ft][ib][base:base + 64, :],
                            start=True, stop=True,
                            tile_position=(base, 0),
                        )
                    if _offload(seq_idx, jc, jj):
                        nc.vector._custom_dve(
                            exp_op,
                            out=at[:, jj * 2:jj * 2 + 2, :],
                            in0=sc[:, :, :],
                            in1=c1_sb[:, 0:1],
                            s0=PC4, s1=PC3, imm2=PC2,
                        )
                    else:
                        nc.scalar.activation(
                            at[:, jj * 2:jj * 2 + 2, :],
                            sc[:, :, :],
                            Exp, scale=float(SCALE),
                        )
                return at

            def pv_chunk(pr, pv_ps, at, jc):
                for hh in range(2):
                    h = 2 * pr + hh
                    for jj in range(2):
                        jt = 2 * jc + jj
                        nc.tensor.matmul(
                            pv_ps[hh][0:DK + 1, :],
                            vh_t[jt][:, h, :],
                            at[:, 2 * jj + hh, :],
                            start=(jt == 0), stop=(jt == NST - 1),
                        )

            def finish_ib(pr, ib, pv_ps, with_outproj=False):
                ft = pr
                for hh in range(2):
                    den = small.tile([1, 512], f32, tag="den")
                    nc.vector.tensor_copy(den[:], pv_ps[hh][DK:DK + 1, :])
                    rec = small.tile([1, 512], f32, tag="rec")
                    nc.vector.reciprocal_approx_fast(rec[:], den[:])
                    rb = small.tile([64, 512], f32, tag="rb")
                    nc.gpsimd.partition_broadcast(rb[:], rec[:])
                    nc.vector.tensor_tensor(
                        y_t[ft][ib][hh * 64:hh * 64 + 64, :],
                        pv_ps[hh][0:DK, :],
                        rb[:],
                        mult,
                    )
                if with_outproj:
                    outproj_sb(ib)

            def outproj_sb(sb):
                for et in range(NDT):
                    po = ps.tile([128, 512], f32, tag="w1", bufs=4,
                                 name=f"po{et}{sb}")
                    for ft in range(2):
                        nc.tensor.matmul(
                            po[:],
                            wo_sb[:, ft, et * 128:(et + 1) * 128],
                            y_t[ft][sb][:],
                            start=(ft == 0), stop=(ft == 1),
                        )
                    o_sb = ostage.tile([128, 512], bf16, tag="ost", name=f"os{et}{sb}")
                    nc.vector.tensor_copy(o_sb[:], po[:])
                    nc.sync.dma_start(
                        out.ap()[et * 128:(et + 1) * 128, sb * 512:(sb + 1) * 512],
                        o_sb[:],
                    )

            # ---- prefix: minimal work before the first exp ----
            qk_sb(wk_sb, bk_sb, xkt, kh_t, 0, 0)
            qk_sb(wq_sb, bq_sb, xqt, qh_t, 0, 0)

            # per-(seq_idx, jc) projection fill work, placed in consumption
            # order so nothing waits on far-future DMAs
            extras = {
                (0, 1): [lambda: qk_sb(wk_sb, bk_sb, xkt, kh_t, 0, 1)],
                (0, 3): [lambda: qk_sb(wk_sb, bk_sb, xkt, kh_t, 0, 2)],
                (0, 4): [lambda: qk_sb(wq_sb, bq_sb, xqt, qh_t, 0, 1)],
                (0, 5): [lambda: qk_sb(wk_sb, bk_sb, xkt, kh_t, 0, 3)],
                (0, 7): [lambda: qk_sb(wq_sb, bq_sb, xqt, qh_t, 0, 2)],
                (1, 0): [lambda: v_g(0)],
                (1, 1): [lambda: v_g(1)],
                (1, 2): [lambda: v_g(2)],
                (1, 3): [lambda: v_g(3)],
                (1, 4): [lambda: v_g(4)],
                (1, 5): [lambda: v_g(5)],
                (1, 6): [lambda: v_g(6)],
                (1, 7): [lambda: v_g(7)],
                (2, 0): [lambda: qk_sb(wq_sb, bq_sb, xqt, qh_t, 0, 3)],
                (2, 2): [lambda: qk_sb(wk_sb, bk_sb, xkt, kh_t, 1, 0)],
                (2, 4): [lambda: qk_sb(wk_sb, bk_sb, xkt, kh_t, 1, 1)],
                (2, 6): [lambda: qk_sb(wq_sb, bq_sb, xqt, qh_t, 1, 0)],
                (3, 0): [lambda: qk_sb(wk_sb, bk_sb, xkt, kh_t, 1, 2)],
                (3, 2): [lambda: qk_sb(wk_sb, bk_sb, xkt, kh_t, 1, 3)],
                (3, 4): [lambda: qk_sb(wq_sb, bq_sb, xqt, qh_t, 1, 1)],
                (3, 6): [lambda: qk_sb(wq_sb, bq_sb, xqt, qh_t, 1, 2)],
                (4, 0): [lambda: qk_sb(wq_sb, bq_sb, xqt, qh_t, 1, 3)],
            }

            seq = [(0, 0), (0, 1), (0, 2), (0, 3), (1, 0), (1, 1), (1, 2), (1, 3)]
            prev = None  # (pr, ib, at_list)
            ppv = None
            for seq_idx, (pr, ib) in enumerate(seq):
                is_last = seq_idx == len(seq) - 1
                at_list = []
                lpv = None
                for jc in range(NST // 2):
                    at_list.append(scores_chunk(pr, ib, jc, seq_idx))
                    for fn in extras.get((seq_idx, jc), ()):
                        fn()
                    if prev is not None:
                        if jc == 0:
                            ppv = [ps.tile([128, 512], f32, tag="w1", bufs=4,
                                           name=f"pv{prev[0]}{prev[1]}_{i}")
                                   for i in range(2)]
                        pv_chunk(prev[0], ppv, prev[2][jc], jc)
                    if is_last and jc >= 2:
                        if jc == 2:
                            lpv = [ps.tile([128, 512], f32, tag="w1", bufs=4,
                                           name=f"pvlast{i}") for i in range(2)]
                        pv_chunk(pr, lpv, at_list[jc - 2], jc - 2)
                if prev is not None:
                    finish_ib(prev[0], prev[1], ppv, with_outproj=(prev[0] == 1))
                prev = (pr, ib, at_list)
                if is_last:
                    # drain the final two pipelined chunks, then finish
                    pv_chunk(pr, lpv, at_list[6], 6)
                    pv_chunk(pr, lpv, at_list[7], 7)
                    finish_ib(pr, ib, lpv, with_outproj=True)

    nc.compile()
    return nc


def _get_nc():
    if "nc" not in _compiled:
        _compiled["nc"] = _build()
    return _compiled["nc"]


def kernel(q, k, v, Wq, bq, Wk, bk, Wv, bv, Wo, bo):
    outp, _ = _run(q, k, v, Wq, bq, Wk, bk, Wv, bv, Wo, bo)
    return outp


def _run(q, k, v, Wq, bq, Wk, bk, Wv, bv, Wo, bo, **run_kwargs):
    from concourse.bass_utils import run_bass_kernel_spmd

    nc = _get_nc()

    q = np.asarray(q, np.float32)
    k = np.asarray(k, np.float32)
    v = np.asarray(v, np.float32)
    Wq = np.asarray(Wq, np.float32)
    Wk = np.asarray(Wk, np.float32)
    Wv = np.asarray(Wv, np.float32)
    Wo = np.asarray(Wo, np.float32)
    bq = np.asarray(bq, np.float32)
    bk = np.asarray(bk, np.float32)
    bv = np.asarray(bv, np.float32)
    bo = np.asarray(bo, np.float32)

    import ml_dtypes
    bf = ml_dtypes.bfloat16
    xqT = [np.ascontiguousarray(q[b].T).astype(bf) for b in range(B)]
    xkT = [np.ascontiguousarray(k[b].T).astype(bf) for b in range(B)]
    xvT = [np.ascontiguousarray(v[b].T).astype(bf) for b in range(B)]

    in_maps = []
    for c in range(NCORES):
        b, hg = divmod(c, HG)
        rows = slice(hg * F, (hg + 1) * F)
        in_maps.append({
            "xq": xqT[b], "xk": xkT[b], "xv": xvT[b],
            "wq": np.ascontiguousarray(Wq[rows].T).astype(bf),
            "wk": np.ascontiguousarray(Wk[rows].T).astype(bf),
            "wv": np.ascontiguousarray(Wv[rows].T).astype(bf),
            "wo": np.ascontiguousarray(Wo[:, rows].T).astype(bf),
            "bq": np.ascontiguousarray(bq[rows].reshape(2, 128).T),
            "bk": np.ascontiguousarray(bk[rows].reshape(2, 128).T),
            "bv": np.ascontiguousarray(bv[rows].reshape(1, F)),
        })

    res = run_bass_kernel_spmd(nc, in_maps, core_ids=list(range(NCORES)), **run_kwargs)

    outp = np.empty((B, S, D), np.float32)
    for b in range(B):
        acc = res.results[b * HG]["out"].astype(np.float32)
        for hg in range(1, HG):
            acc = acc + res.results[b * HG + hg]["out"].astype(np.float32)
        outp[b] = acc.T + bo[None, :]
    return outp, res


# revision 12
# speedup vs baseline: 1.9532x; 1.9532x over previous
"""Multi-head self-attention (B=2, S=2048, D=1024, H=16) on 8 TRN2 NeuronCores.

Sharding: data-parallel over batch (2) x tensor-parallel over head-groups (4).
Core c = b*4 + hg handles batch b, heads hg*4..hg*4+3 (4 heads, 256 features).

Per-core device program (SPMD, identical on all cores):
  - QKV projections for the core's 256 output features (column-parallel)
  - full S x S attention for its 4 heads (softmax without max-subtraction,
    denominators via an appended ones-column in the PV matmul)
  - partial output projection (row-parallel): out_partial^T [1024, 2048]
Host: shards/transposes inputs, sums the 4 partial outputs per batch
(the "all-reduce"), adds bo, and untransposes.

Performance structure (v2):
  - The Scalar engine's exp() is the core bottleneck (16.8M elements/core at
    1 elem/cycle/lane). A fraction of exp tiles is offloaded to the Vector
    engine via a one-instruction Schraudolph bit-trick (tensor_scalar to
    int16 aliased as bf16; ~3% max elementwise error that softmax mostly
    cancels -- measured end-to-end contribution ~7e-3).
  - Inputs stream s-block-major so the first scores/exp start ~10us in
    instead of ~50us; projection matmuls are interleaved chunk-by-chunk into
    the attention loop so the PE fills exp-paced gaps without stalling ACT.
  - 8 junk matmuls at t=0 warm the PE HAM clock gate during the DMA prefix.
"""

import numpy as np

B, S, D = 2, 2048, 1024
H, DK = 16, 64
NCORES = 8
HG = 4          # head groups (tensor parallel)
HPG = 4         # heads per group
F = HPG * DK    # 256 local features per core
SCALE = 1.0 / np.sqrt(DK)

# Schraudolph exp on the Vector engine via one stock tensor_scalar:
# bf16 bits = round(s * SCH_A + SCH_B) written as int16 into a bf16-aliased
# at tile. SCH_A = 0.125*log2(e)*128 maps the raw score onto the bf16
# exponent/mantissa fixed point; SCH_B centers the linear-interp error
# (max ~3%; softmax normalization cancels most of it; scores span +-75 so
# a polynomial can't cover the range, but this handles all of it).
SCH_A = float(0.125 * np.log2(np.e) * 128.0)
SCH_B = float((127.0 - 0.0434) * 128.0)

_compiled = {}


import os
_OFFLOAD_ON = os.environ.get("KERNEL_EXP_OFFLOAD", "1") == "1"


def _offload(seq_idx, jc, jj):
    """Which exp units go to the Vector engine instead of ScalarE."""
    return _OFFLOAD_ON and seq_idx >= 1 and jj == 1 and jc in (1, 3, 5)


def _build():
    import concourse.bacc as bacc
    import concourse.tile as tile
    from concourse import mybir

    f32 = mybir.dt.float32
    bf16 = mybir.dt.bfloat16
    Exp = mybir.ActivationFunctionType.Exp
    mult = mybir.AluOpType.mult

    nc = bacc.Bacc("TRN2", target_bir_lowering=False, debug=False,
                   enable_asserts=True, num_devices=NCORES)

    xq = nc.dram_tensor("xq", (D, S), bf16, kind="ExternalInput")   # q[b].T
    xk = nc.dram_tensor("xk", (D, S), bf16, kind="ExternalInput")
    xv = nc.dram_tensor("xv", (D, S), bf16, kind="ExternalInput")
    wq = nc.dram_tensor("wq", (D, F), bf16, kind="ExternalInput")   # Wq[rows].T
    wk = nc.dram_tensor("wk", (D, F), bf16, kind="ExternalInput")
    wv = nc.dram_tensor("wv", (D, F), bf16, kind="ExternalInput")
    wo = nc.dram_tensor("wo", (F, D), bf16, kind="ExternalInput")   # Wo[:, cols].T
    bq = nc.dram_tensor("bq", (128, 2), f32, kind="ExternalInput")  # bias, f-tiled
    bk = nc.dram_tensor("bk", (128, 2), f32, kind="ExternalInput")
    bv = nc.dram_tensor("bv", (1, F), f32, kind="ExternalInput")
    out = nc.dram_tensor("out", (D, S), bf16, kind="ExternalOutput")  # partial^T

    NDT = D // 128   # 8 d-tiles
    NST = S // 128   # 16 s-tiles (j tiles)
    NSB = S // 512   # 4 s-blocks (i blocks)

    with tile.TileContext(nc) as tc:
        import contextlib
        with contextlib.ExitStack() as ctx:
            consts = ctx.enter_context(tc.tile_pool(name="consts", bufs=1))
            big = ctx.enter_context(tc.tile_pool(name="big", bufs=25))
            atp = ctx.enter_context(tc.tile_pool(name="atp", bufs=9))
            acts = ctx.enter_context(tc.tile_pool(name="acts", bufs=1))
            ostage = ctx.enter_context(tc.tile_pool(name="ostage", bufs=3))
            small = ctx.enter_context(tc.tile_pool(name="small", bufs=2))
            ps = ctx.enter_context(tc.tile_pool(name="ps", bufs=1, space="PSUM"))

            from concourse.tile_rust import add_dep_helper

            # ---- SBUF constants / persistent tiles ----
            wq_sb = consts.tile([128, NDT, F], bf16, tag="wq")
            wk_sb = consts.tile([128, NDT, F], bf16, tag="wk")
            wv_sb = consts.tile([128, NDT, F], bf16, tag="wv")
            wo_sb = consts.tile([128, 2, D], bf16, tag="wo")
            bq_sb = consts.tile([128, 2], f32, tag="bq")
            bk_sb = consts.tile([128, 2], f32, tag="bk")
            bv_sb = consts.tile([128, F], f32, tag="bv")
            junk = consts.tile([128, 512], bf16, tag="junk")
            nc.vector.memset(junk[:], 0.125)

            # persistent activations
            qh_t = [[acts.tile([128, 512], bf16, tag=f"qh{ft}{sb}", name=f"qh{ft}{sb}")
                     for sb in range(NSB)] for ft in range(2)]
            kh_t = [[acts.tile([128, 512], bf16, tag=f"kh{ft}{sb}", name=f"kh{ft}{sb}")
                     for sb in range(NSB)] for ft in range(2)]
            vh_t = [acts.tile([128, HPG, DK + 1], bf16, tag=f"vh{st}", name=f"vh{st}")
                    for st in range(NST)]
            for st in range(NST):
                nc.vector.memset(vh_t[st][:, :, DK:DK + 1], 1.0)
            y_t = [[acts.tile([128, 512], bf16, tag=f"y{ft}{sb}", name=f"y{ft}{sb}")
                    for sb in range(NSB)] for ft in range(2)]

            # ---- HAM warmup: keep the PE busy during the DMA prefix so the
            # clock gate is at 8/8 when real matmuls arrive ----
            wps = ps.tile([128, 512], f32, tag="w1", bufs=4, name="warm")
            for i in range(8):
                nc.tensor.matmul(wps[:], junk[:, 0:128], junk[:],
                                 start=(i == 0), stop=(i == 7))
            nc.vector.tensor_copy(junk[:, 0:64], wps[:, 0:64])  # keep it live

            # ---- input DMAs, consumption order, chained so the queue
            # services them in exactly this order ----
            xqt = [big.tile([128, S], bf16, tag="big", name=f"xq{dt}") for dt in range(NDT)]
            xkt = [big.tile([128, S], bf16, tag="big", name=f"xk{dt}") for dt in range(NDT)]
            xvt = [big.tile([128, S], bf16, tag="big", name=f"xv{dt}") for dt in range(NDT)]

            # DMA issue is ~0.6us per descriptor on an engine queue, so spread
            # issue across idle engines: Scalar takes wk/wq (done before its
            # first exp), GpSimd takes biases+xq, Sync streams xk then xv.
            chain = {"prev": None, "first_of_group": None}

            def dma(dst, src):
                d = nc.sync.dma_start(dst, src)
                if chain["first_of_group"] is None and chain["prev"] is not None:
                    add_dep_helper(d.ins, chain["prev"].ins, sync=True,
                                   reason="input DMA group ordering")
                    chain["first_of_group"] = d
                chain["last"] = d
                return d

            def group_end():
                chain["prev"] = chain.get("last")
                chain["first_of_group"] = None

            def load_w(eng, w_sb_, wdram):
                for dt in range(NDT):
                    eng.dma_start(w_sb_[:, dt, :],
                                  wdram.ap()[dt * 128:(dt + 1) * 128, :])

            def load_x(ts, xdram, s0, s1, eng=None):
                for dt in range(NDT):
                    dst = ts[dt][:, s0:s1]
                    src = xdram.ap()[dt * 128:(dt + 1) * 128, s0:s1]
                    if eng is None:
                        dma(dst, src)
                    else:
                        eng.dma_start(dst, src)

            load_w(nc.scalar, wk_sb, wk)
            load_w(nc.scalar, wq_sb, wq)
            nc.gpsimd.dma_start(bq_sb[:], bq.ap()[:])
            nc.gpsimd.dma_start(bk_sb[:], bk.ap()[:])
            nc.gpsimd.dma_start(bv_sb[:], bv.ap().to_broadcast((128, F)))
            load_x(xqt, xq, 0, 512, eng=nc.gpsimd)
            load_x(xqt, xq, 512, 1024, eng=nc.gpsimd)
            load_x(xqt, xq, 1024, 2048, eng=nc.gpsimd)
            load_w(nc.gpsimd, wv_sb, wv)
            for ft in range(2):
                nc.gpsimd.dma_start(wo_sb[:, ft, :],
                                    wo.ap()[ft * 128:(ft + 1) * 128, :])
            load_x(xkt, xk, 0, 512)
            group_end()
            load_x(xkt, xk, 512, 1024)
            group_end()
            load_x(xkt, xk, 1024, 2048)
            group_end()
            load_x(xvt, xv, 0, 1024)
            group_end()
            load_x(xvt, xv, 1024, 2048)

            # ---- projection helpers ----
            def qk_sb(w_sb_, b_sb_, xts, dst, ft, sb):
                acc = ps.tile([128, 512], f32, tag="w1", bufs=4,
                              name=f"qk{ft}{sb}")
                for dt in range(NDT):
                    nc.tensor.matmul(
                        acc[:],
                        w_sb_[:, dt, ft * 128:(ft + 1) * 128],
                        xts[dt][:, sb * 512:(sb + 1) * 512],
                        start=(dt == 0), stop=(dt == NDT - 1),
                    )
                nc.vector.tensor_scalar_add(dst[ft][sb][:], acc[:],
                                            b_sb_[:, ft:ft + 1])

            def v_g(g):
                accs = [ps.tile([128, 512], f32, tag="w1", bufs=4,
                                name=f"vps{g}{j}") for j in range(2)]
                for dt in range(NDT):
                    for j in range(2):
                        st = 2 * g + j
                        nc.tensor.matmul(
                            accs[j][:, 0:F],
                            xvt[dt][:, st * 128:(st + 1) * 128],
                            wv_sb[:, dt, :],
                            start=(dt == 0), stop=(dt == NDT - 1),
                        )
                for j in range(2):
                    st = 2 * g + j
                    nc.vector.tensor_tensor(
                        vh_t[st][:, :, 0:DK],
                        accs[j][:, 0:F].rearrange("p (h c) -> p h c", h=HPG),
                        bv_sb[:].rearrange("p (h c) -> p h c", h=HPG),
                        mybir.AluOpType.add,
                    )

            # ---- attention chunk helpers ----
            def scores_chunk(pr, ib, jc, seq_idx):
                ft = pr
                at = atp.tile([128, 4, 512], bf16, tag="at", name=f"at{pr}{ib}{jc}")
                for jj in range(2):
                    jt = jc * 2 + jj
                    sc = ps.tile([128, 2, 512], f32, tag="w2", bufs=2, name="sc")
                    for hh in range(2):
                        base = hh * 64
                        nc.tensor.matmul(
                            sc[:, hh, :],
                            kh_t[ft][jt // 4][base:base + 64,
                                              (jt % 4) * 128:(jt % 4 + 1) * 128],
                            qh_t[ft][ib][base:base + 64, :],
                            start=True, stop=True,
                            tile_position=(base, 0),
                        )
                    if _offload(seq_idx, jc, jj):
                        nc.vector.tensor_scalar(
                            at[:, jj * 2:jj * 2 + 2, :].bitcast(mybir.dt.int16),
                            sc[:, :, :],
                            SCH_A, SCH_B,
                            mybir.AluOpType.mult, mybir.AluOpType.add,
                        )
                    else:
                        nc.scalar.activation(
                            at[:, jj * 2:jj * 2 + 2, :],
                            sc[:, :, :],
                            Exp, scale=float(SCALE),
                        )
                return at

            def pv_chunk(pr, pv_ps, at, jc):
                for hh in range(2):
                    h = 2 * pr + hh
                    for jj in range(2):
                        jt = 2 * jc + jj
                        nc.tensor.matmul(
                            pv_ps[hh][0:DK + 1, :],
                            vh_t[jt][:, h, :],
                            at[:, 2 * jj + hh, :],
                            start=(jt == 0), stop=(jt == NST - 1),
                        )

            def finish_ib(pr, ib, pv_ps, with_outproj=False):
                ft = pr
                for hh in range(2):
                    den = small.tile([1, 512], f32, tag="den")
                    nc.vector.tensor_copy(den[:], pv_ps[hh][DK:DK + 1, :])
                    rec = small.tile([1, 512], f32, tag="rec")
                    nc.vector.reciprocal_approx_fast(rec[:], den[:])
                    rb = small.tile([64, 512], f32, tag="rb")
                    nc.gpsimd.partition_broadcast(rb[:], rec[:])
                    nc.vector.tensor_tensor(
                        y_t[ft][ib][hh * 64:hh * 64 + 64, :],
                        pv_ps[hh][0:DK, :],
                        rb[:],
                        mult,
                    )
                if with_outproj:
                    outproj_sb(ib)

            def outproj_sb(sb):
                for et in range(NDT):
                    po = ps.tile([128, 512], f32, tag="w1", bufs=4,
                                 name=f"po{et}{sb}")
                    for ft in range(2):
                        nc.tensor.matmul(
                            po[:],
                            wo_sb[:, ft, et * 128:(et + 1) * 128],
                            y_t[ft][sb][:],
                            start=(ft == 0), stop=(ft == 1),
                        )
                    o_sb = ostage.tile([128, 512], bf16, tag="ost", name=f"os{et}{sb}")
                    nc.vector.tensor_copy(o_sb[:], po[:])
                    nc.sync.dma_start(
                        out.ap()[et * 128:(et + 1) * 128, sb * 512:(sb + 1) * 512],
                        o_sb[:],
                    )

            # ---- prefix: minimal work before the first exp ----
            qk_sb(wk_sb, bk_sb, xkt, kh_t, 0, 0)
            qk_sb(wq_sb, bq_sb, xqt, qh_t, 0, 0)

            # per-(seq_idx, jc) projection fill work, placed in consumption
            # order so nothing waits on far-future DMAs
            extras = {
                (0, 1): [lambda: qk_sb(wk_sb, bk_sb, xkt, kh_t, 0, 1)],
                (0, 3): [lambda: qk_sb(wk_sb, bk_sb, xkt, kh_t, 0, 2)],
                (0, 4): [lambda: qk_sb(wq_sb, bq_sb, xqt, qh_t, 0, 1)],
                (0, 5): [lambda: qk_sb(wk_sb, bk_sb, xkt, kh_t, 0, 3)],
                (0, 7): [lambda: qk_sb(wq_sb, bq_sb, xqt, qh_t, 0, 2)],
                (1, 0): [lambda: v_g(0)],
                (1, 1): [lambda: v_g(1)],
                (1, 2): [lambda: v_g(2)],
                (1, 3): [lambda: v_g(3)],
                (1, 4): [lambda: v_g(4)],
                (1, 5): [lambda: v_g(5)],
                (1, 6): [lambda: v_g(6)],
                (1, 7): [lambda: v_g(7)],
                (2, 0): [lambda: qk_sb(wq_sb, bq_sb, xqt, qh_t, 0, 3)],
                (2, 2): [lambda: qk_sb(wk_sb, bk_sb, xkt, kh_t, 1, 0)],
                (2, 4): [lambda: qk_sb(wk_sb, bk_sb, xkt, kh_t, 1, 1)],
                (2, 6): [lambda: qk_sb(wq_sb, bq_sb, xqt, qh_t, 1, 0)],
                (3, 0): [lambda: qk_sb(wk_sb, bk_sb, xkt, kh_t, 1, 2)],
                (3, 2): [lambda: qk_sb(wk_sb, bk_sb, xkt, kh_t, 1, 3)],
                (3, 4): [lambda: qk_sb(wq_sb, bq_sb, xqt, qh_t, 1, 1)],
                (3, 6): [lambda: qk_sb(wq_sb, bq_sb, xqt, qh_t, 1, 2)],
                (4, 0): [lambda: qk_sb(wq_sb, bq_sb, xqt, qh_t, 1, 3)],
            }

            seq = [(0, 0), (0, 1), (0, 2), (0, 3), (1, 0), (1, 1), (1, 2), (1, 3)]
            prev = None  # (pr, ib, at_list)
            ppv = None
            for seq_idx, (pr, ib) in enumerate(seq):
                is_last = seq_idx == len(seq) - 1
                at_list = []
                lpv = None
                for jc in range(NST // 2):
                    at_list.append(scores_chunk(pr, ib, jc, seq_idx))
                    for fn in extras.get((seq_idx, jc), ()):
                        fn()
                    if prev is not None:
                        if jc == 0:
                            ppv = [ps.tile([128, 512], f32, tag="w1", bufs=4,
                                           name=f"pv{prev[0]}{prev[1]}_{i}")
                                   for i in range(2)]
                        pv_chunk(prev[0], ppv, prev[2][jc], jc)
                    if is_last and jc >= 2:
                        if jc == 2:
                            lpv = [ps.tile([128, 512], f32, tag="w1", bufs=4,
                                           name=f"pvlast{i}") for i in range(2)]
                        pv_chunk(pr, lpv, at_list[jc - 2], jc - 2)
                if prev is not None:
                    finish_ib(prev[0], prev[1], ppv, with_outproj=(prev[0] == 1))
                prev = (pr, ib, at_list)
                if is_last:
                    # drain the final two pipelined chunks, then finish
                    pv_chunk(pr, lpv, at_list[6], 6)
                    pv_chunk(pr, lpv, at_list[7], 7)
                    finish_ib(pr, ib, lpv, with_outproj=True)

    nc.compile()
    return nc


def _get_nc():
    if "nc" not in _compiled:
        _compiled["nc"] = _build()
    return _compiled["nc"]


def kernel(q, k, v, Wq, bq, Wk, bk, Wv, bv, Wo, bo):
    outp, _ = _run(q, k, v, Wq, bq, Wk, bk, Wv, bv, Wo, bo)
    return outp


def _run(q, k, v, Wq, bq, Wk, bk, Wv, bv, Wo, bo, **run_kwargs):
    from concourse.bass_utils import run_bass_kernel_spmd

    nc = _get_nc()

    q = np.asarray(q, np.float32)
    k = np.asarray(k, np.float32)
    v = np.asarray(v, np.float32)
    Wq = np.asarray(Wq, np.float32)
    Wk = np.asarray(Wk, np.float32)
    Wv = np.asarray(Wv, np.float32)
    Wo = np.asarray(Wo, np.float32)
    bq = np.asarray(bq, np.float32)
    bk = np.asarray(bk, np.float32)
    bv = np.asarray(bv, np.float32)
    bo = np.asarray(bo, np.float32)

    import ml_dtypes
    bf = ml_dtypes.bfloat16
    xqT = [np.ascontiguousarray(q[b].T).astype(bf) for b in range(B)]
    xkT = [np.ascontiguousarray(k[b].T).astype(bf) for b in range(B)]
    xvT = [np.ascontiguousarray(v[b].T).astype(bf) for b in range(B)]

    in_maps = []
    for c in range(NCORES):
        b, hg = divmod(c, HG)
        rows = slice(hg * F, (hg + 1) * F)
        in_maps.append({
            "xq": xqT[b], "xk": xkT[b], "xv": xvT[b],
            "wq": np.ascontiguousarray(Wq[rows].T).astype(bf),
            "wk": np.ascontiguousarray(Wk[rows].T).astype(bf),
            "wv": np.ascontiguousarray(Wv[rows].T).astype(bf),
            "wo": np.ascontiguousarray(Wo[:, rows].T).astype(bf),
            "bq": np.ascontiguousarray(bq[rows].reshape(2, 128).T),
            "bk": np.ascontiguousarray(bk[rows].reshape(2, 128).T),
            "bv": np.ascontiguousarray(bv[rows].reshape(1, F)),
        })

    res = run_bass_kernel_spmd(nc, in_maps, core_ids=list(range(NCORES)), **run_kwargs)

    outp = np.empty((B, S, D), np.float32)
    for b in range(B):
        acc = res.results[b * HG]["out"].astype(np.float32)
        for hg in range(1, HG):
            acc = acc + res.results[b * HG + hg]["out"].astype(np.float32)
        outp[b] = acc.T + bo[None, :]
    return outp, res


# revision 14
# speedup vs baseline: 2.0059x; 1.0270x over previous
"""Multi-head self-attention (B=2, S=2048, D=1024, H=16) on 8 TRN2 NeuronCores.

Sharding: data-parallel over batch (2) x tensor-parallel over head-groups (4).
Core c = b*4 + hg handles batch b, heads hg*4..hg*4+3 (4 heads, 256 features).

Per-core device program (SPMD, identical on all cores):
  - QKV projections for the core's 256 output features (column-parallel)
  - full S x S attention for its 4 heads (softmax without max-subtraction,
    denominators via an appended ones-column in the PV matmul)
  - partial output projection (row-parallel): out_partial^T [1024, 2048]
Host: shards/transposes inputs, sums the 4 partial outputs per batch
(the "all-reduce"), adds bo, and untransposes.

Performance structure (v2):
  - The Scalar engine's exp() is the core bottleneck (16.8M elements/core at
    1 elem/cycle/lane). A fraction of exp tiles is offloaded to the Vector
    engine via a one-instruction Schraudolph bit-trick (tensor_scalar to
    int16 aliased as bf16; ~3% max elementwise error that softmax mostly
    cancels -- measured end-to-end contribution ~7e-3).
  - Inputs stream s-block-major so the first scores/exp start ~10us in
    instead of ~50us; projection matmuls are interleaved chunk-by-chunk into
    the attention loop so the PE fills exp-paced gaps without stalling ACT.
  - 8 junk matmuls at t=0 warm the PE HAM clock gate during the DMA prefix.
"""

import numpy as np

B, S, D = 2, 2048, 1024
H, DK = 16, 64
NCORES = 8
HG = 4          # head groups (tensor parallel)
HPG = 4         # heads per group
F = HPG * DK    # 256 local features per core
SCALE = 1.0 / np.sqrt(DK)

# Schraudolph exp on the Vector engine via one stock tensor_scalar:
# bf16 bits = round(s * SCH_A + SCH_B) written as int16 into a bf16-aliased
# at tile. SCH_A = 0.125*log2(e)*128 maps the raw score onto the bf16
# exponent/mantissa fixed point; SCH_B centers the linear-interp error
# (max ~3%; softmax normalization cancels most of it; scores span +-75 so
# a polynomial can't cover the range, but this handles all of it).
SCH_A = float(0.125 * np.log2(np.e) * 128.0)
SCH_B = float((127.0 - 0.0434) * 128.0)

_compiled = {}


import os
_OFFLOAD_ON = os.environ.get("KERNEL_EXP_OFFLOAD", "1") == "1"


def _offload(seq_idx, jc, jj):
    """Which exp units go to the Vector engine instead of ScalarE."""
    return _OFFLOAD_ON and seq_idx >= 1 and jj == 1 and jc in (1, 3, 5)


def _build():
    import concourse.bacc as bacc
    import concourse.tile as tile
    from concourse import mybir

    f32 = mybir.dt.float32
    bf16 = mybir.dt.bfloat16
    Exp = mybir.ActivationFunctionType.Exp
    mult = mybir.AluOpType.mult

    nc = bacc.Bacc("TRN2", target_bir_lowering=False, debug=False,
                   enable_asserts=True, num_devices=NCORES)

    xq = nc.dram_tensor("xq", (D, S), bf16, kind="ExternalInput")   # q[b].T
    xk = nc.dram_tensor("xk", (D, S), bf16, kind="ExternalInput")
    xv = nc.dram_tensor("xv", (D, S), bf16, kind="ExternalInput")
    wq = nc.dram_tensor("wq", (D, F), bf16, kind="ExternalInput")   # Wq[rows].T
    wk = nc.dram_tensor("wk", (D, F), bf16, kind="ExternalInput")
    wv = nc.dram_tensor("wv", (D, F), bf16, kind="ExternalInput")
    wo = nc.dram_tensor("wo", (F, D), bf16, kind="ExternalInput")   # Wo[:, cols].T
    bq = nc.dram_tensor("bq", (128, 2), f32, kind="ExternalInput")  # bias, f-tiled
    bk = nc.dram_tensor("bk", (128, 2), f32, kind="ExternalInput")
    bv = nc.dram_tensor("bv", (1, F), f32, kind="ExternalInput")
    out = nc.dram_tensor("out", (D, S), bf16, kind="ExternalOutput")  # partial^T

    NDT = D // 128   # 8 d-tiles
    NST = S // 128   # 16 s-tiles (j tiles)
    NSB = S // 512   # 4 s-blocks (i blocks)

    with tile.TileContext(nc) as tc:
        import contextlib
        with contextlib.ExitStack() as ctx:
            consts = ctx.enter_context(tc.tile_pool(name="consts", bufs=1))
            big = ctx.enter_context(tc.tile_pool(name="big", bufs=25))
            atp = ctx.enter_context(tc.tile_pool(name="atp", bufs=9))
            acts = ctx.enter_context(tc.tile_pool(name="acts", bufs=1))
            ostage = ctx.enter_context(tc.tile_pool(name="ostage", bufs=3))
            small = ctx.enter_context(tc.tile_pool(name="small", bufs=2))
            ps = ctx.enter_context(tc.tile_pool(name="ps", bufs=1, space="PSUM"))

            from concourse.tile_rust import add_dep_helper

            # ---- SBUF constants / persistent tiles ----
            wq_sb = consts.tile([128, NDT, F], bf16, tag="wq")
            wk_sb = consts.tile([128, NDT, F], bf16, tag="wk")
            wv_sb = consts.tile([128, NDT, F], bf16, tag="wv")
            wo_sb = consts.tile([128, 2, D], bf16, tag="wo")
            bq_sb = consts.tile([128, 2], f32, tag="bq")
            bk_sb = consts.tile([128, 2], f32, tag="bk")
            bv_sb = consts.tile([128, F], f32, tag="bv")
            junk = consts.tile([128, 512], bf16, tag="junk")
            nc.vector.memset(junk[:], 0.125)

            # persistent activations
            qh_t = [[acts.tile([128, 512], bf16, tag=f"qh{ft}{sb}", name=f"qh{ft}{sb}")
                     for sb in range(NSB)] for ft in range(2)]
            kh_t = [[acts.tile([128, 512], bf16, tag=f"kh{ft}{sb}", name=f"kh{ft}{sb}")
                     for sb in range(NSB)] for ft in range(2)]
            vh_t = [acts.tile([128, HPG, DK + 1], bf16, tag=f"vh{st}", name=f"vh{st}")
                    for st in range(NST)]
            for st in range(NST):
                nc.vector.memset(vh_t[st][:, :, DK:DK + 1], 1.0)
            y_t = [[acts.tile([128, 512], bf16, tag=f"y{ft}{sb}", name=f"y{ft}{sb}")
                    for sb in range(NSB)] for ft in range(2)]

            # ---- HAM warmup: keep the PE busy during the DMA prefix so the
            # clock gate is at 8/8 when real matmuls arrive ----
            wps = ps.tile([128, 512], f32, tag="w1", bufs=4, name="warm")
            for i in range(20):
                nc.tensor.matmul(wps[:], junk[:, 0:128], junk[:],
                                 start=(i == 0), stop=(i == 19))
            nc.vector.tensor_copy(junk[:, 0:64], wps[:, 0:64])  # keep it live

            # ---- input DMAs, consumption order, chained so the queue
            # services them in exactly this order ----
            xqt = [big.tile([128, S], bf16, tag="big", name=f"xq{dt}") for dt in range(NDT)]
            xkt = [big.tile([128, S], bf16, tag="big", name=f"xk{dt}") for dt in range(NDT)]
            xvt = [big.tile([128, S], bf16, tag="big", name=f"xv{dt}") for dt in range(NDT)]

            # DMA issue is ~0.6us per descriptor on an engine queue, so spread
            # issue across idle engines: Scalar takes wk/wq (done before its
            # first exp), GpSimd takes biases+xq, Sync streams xk then xv.
            chain = {"prev": None, "first_of_group": None}

            def dma(dst, src):
                d = nc.sync.dma_start(dst, src)
                if chain["first_of_group"] is None and chain["prev"] is not None:
                    add_dep_helper(d.ins, chain["prev"].ins, sync=True,
                                   reason="input DMA group ordering")
                    chain["first_of_group"] = d
                chain["last"] = d
                return d

            def group_end():
                chain["prev"] = chain.get("last")
                chain["first_of_group"] = None

            def load_w(eng, w_sb_, wdram):
                for dt in range(NDT):
                    eng.dma_start(w_sb_[:, dt, :],
                                  wdram.ap()[dt * 128:(dt + 1) * 128, :])

            def load_x(ts, xdram, s0, s1, eng=None):
                for dt in range(NDT):
                    dst = ts[dt][:, s0:s1]
                    src = xdram.ap()[dt * 128:(dt + 1) * 128, s0:s1]
                    if eng is None:
                        dma(dst, src)
                    else:
                        eng.dma_start(dst, src)

            def load_w_ft(eng, w_sb_, wdram, ft):
                for dt in range(NDT):
                    eng.dma_start(
                        w_sb_[:, dt, ft * 128:(ft + 1) * 128],
                        wdram.ap()[dt * 128:(dt + 1) * 128,
                                   ft * 128:(ft + 1) * 128])

            # critical-first: only the ~3MB needed by the first scores chunk
            # goes ahead of everything (DMA bandwidth, not issue rate, is the
            # prefix limit)
            load_w_ft(nc.scalar, wk_sb, wk, 0)
            load_w_ft(nc.scalar, wq_sb, wq, 0)
            nc.gpsimd.dma_start(bq_sb[:], bq.ap()[:])
            nc.gpsimd.dma_start(bk_sb[:], bk.ap()[:])
            load_x(xqt, xq, 0, 512, eng=nc.gpsimd)
            load_w(nc.gpsimd, wv_sb, wv)
            nc.gpsimd.dma_start(bv_sb[:], bv.ap().to_broadcast((128, F)))
            load_x(xqt, xq, 512, 1024, eng=nc.gpsimd)
            load_w_ft(nc.gpsimd, wk_sb, wk, 1)
            load_w_ft(nc.gpsimd, wq_sb, wq, 1)
            load_x(xqt, xq, 1024, 2048, eng=nc.gpsimd)
            for ft in range(2):
                nc.gpsimd.dma_start(wo_sb[:, ft, :],
                                    wo.ap()[ft * 128:(ft + 1) * 128, :])
            load_x(xkt, xk, 0, 512)
            group_end()
            load_x(xkt, xk, 512, 1024)
            group_end()
            load_x(xkt, xk, 1024, 2048)
            group_end()
            load_x(xvt, xv, 0, 1024)
            group_end()
            load_x(xvt, xv, 1024, 2048)

            # ---- projection helpers ----
            def qk_sb(w_sb_, b_sb_, xts, dst, ft, sb):
                acc = ps.tile([128, 512], f32, tag="w1", bufs=4,
                              name=f"qk{ft}{sb}")
                for dt in range(NDT):
                    nc.tensor.matmul(
                        acc[:],
                        w_sb_[:, dt, ft * 128:(ft + 1) * 128],
                        xts[dt][:, sb * 512:(sb + 1) * 512],
                        start=(dt == 0), stop=(dt == NDT - 1),
                    )
                nc.vector.tensor_scalar_add(dst[ft][sb][:], acc[:],
                                            b_sb_[:, ft:ft + 1])

            def v_g(g):
                accs = [ps.tile([128, 512], f32, tag="w1", bufs=4,
                                name=f"vps{g}{j}") for j in range(2)]
                for dt in range(NDT):
                    for j in range(2):
                        st = 2 * g + j
                        nc.tensor.matmul(
                            accs[j][:, 0:F],
                            xvt[dt][:, st * 128:(st + 1) * 128],
                            wv_sb[:, dt, :],
                            start=(dt == 0), stop=(dt == NDT - 1),
                        )
                for j in range(2):
                    st = 2 * g + j
                    nc.vector.tensor_tensor(
                        vh_t[st][:, :, 0:DK],
                        accs[j][:, 0:F].rearrange("p (h c) -> p h c", h=HPG),
                        bv_sb[:].rearrange("p (h c) -> p h c", h=HPG),
                        mybir.AluOpType.add,
                    )

            # ---- attention chunk helpers ----
            def scores_chunk(pr, ib, jc, seq_idx):
                ft = pr
                at = atp.tile([128, 4, 512], bf16, tag="at", name=f"at{pr}{ib}{jc}")
                for jj in range(2):
                    jt = jc * 2 + jj
                    sc = ps.tile([128, 2, 512], f32, tag="w2", bufs=2, name="sc")
                    for hh in range(2):
                        base = hh * 64
                        nc.tensor.matmul(
                            sc[:, hh, :],
                            kh_t[ft][jt // 4][base:base + 64,
                                              (jt % 4) * 128:(jt % 4 + 1) * 128],
                            qh_t[ft][ib][base:base + 64, :],
                            start=True, stop=True,
                            tile_position=(base, 0),
                        )
                    if _offload(seq_idx, jc, jj):
                        nc.vector.tensor_scalar(
                            at[:, jj * 2:jj * 2 + 2, :].bitcast(mybir.dt.int16),
                            sc[:, :, :],
                            SCH_A, SCH_B,
                            mybir.AluOpType.mult, mybir.AluOpType.add,
                        )
                    else:
                        nc.scalar.activation(
                            at[:, jj * 2:jj * 2 + 2, :],
                            sc[:, :, :],
                            Exp, scale=float(SCALE),
                        )
                return at

            def pv_chunk(pr, pv_ps, at, jc):
                for hh in range(2):
                    h = 2 * pr + hh
                    for jj in range(2):
                        jt = 2 * jc + jj
                        nc.tensor.matmul(
                            pv_ps[hh][0:DK + 1, :],
                            vh_t[jt][:, h, :],
                            at[:, 2 * jj + hh, :],
                            start=(jt == 0), stop=(jt == NST - 1),
                        )

            def finish_ib(pr, ib, pv_ps, with_outproj=False):
                ft = pr
                for hh in range(2):
                    den = small.tile([1, 512], f32, tag="den")
                    nc.vector.tensor_copy(den[:], pv_ps[hh][DK:DK + 1, :])
                    rec = small.tile([1, 512], f32, tag="rec")
                    nc.vector.reciprocal_approx_fast(rec[:], den[:])
                    rb = small.tile([64, 512], f32, tag="rb")
                    nc.gpsimd.partition_broadcast(rb[:], rec[:])
                    nc.vector.tensor_tensor(
                        y_t[ft][ib][hh * 64:hh * 64 + 64, :],
                        pv_ps[hh][0:DK, :],
                        rb[:],
                        mult,
                    )
                if with_outproj:
                    outproj_sb(ib)

            def outproj_et(sb, et):
                po = ps.tile([128, 512], f32, tag="w1", bufs=4,
                             name=f"po{et}{sb}")
                for ft in range(2):
                    nc.tensor.matmul(
                        po[:],
                        wo_sb[:, ft, et * 128:(et + 1) * 128],
                        y_t[ft][sb][:],
                        start=(ft == 0), stop=(ft == 1),
                    )
                o_sb = ostage.tile([128, 512], bf16, tag="ost", name=f"os{et}{sb}")
                nc.vector.tensor_copy(o_sb[:], po[:])
                nc.sync.dma_start(
                    out.ap()[et * 128:(et + 1) * 128, sb * 512:(sb + 1) * 512],
                    o_sb[:],
                )

            def outproj_sb(sb):
                for et in range(NDT):
                    outproj_et(sb, et)

            # ---- prefix: minimal work before the first exp ----
            qk_sb(wk_sb, bk_sb, xkt, kh_t, 0, 0)
            qk_sb(wq_sb, bq_sb, xqt, qh_t, 0, 0)

            # per-(seq_idx, jc) projection fill work, placed in consumption
            # order so nothing waits on far-future DMAs
            extras = {
                (0, 1): [lambda: qk_sb(wk_sb, bk_sb, xkt, kh_t, 0, 1)],
                (0, 3): [lambda: qk_sb(wk_sb, bk_sb, xkt, kh_t, 0, 2)],
                (0, 4): [lambda: qk_sb(wq_sb, bq_sb, xqt, qh_t, 0, 1)],
                (0, 5): [lambda: qk_sb(wk_sb, bk_sb, xkt, kh_t, 0, 3)],
                (0, 7): [lambda: qk_sb(wq_sb, bq_sb, xqt, qh_t, 0, 2)],
                (1, 0): [lambda: v_g(0)],
                (1, 1): [lambda: v_g(1)],
                (1, 2): [lambda: v_g(2)],
                (1, 3): [lambda: v_g(3)],
                (1, 4): [lambda: v_g(4)],
                (1, 5): [lambda: v_g(5)],
                (1, 6): [lambda: v_g(6)],
                (1, 7): [lambda: v_g(7)],
                (2, 0): [lambda: qk_sb(wq_sb, bq_sb, xqt, qh_t, 0, 3)],
                (2, 2): [lambda: qk_sb(wk_sb, bk_sb, xkt, kh_t, 1, 0)],
                (2, 4): [lambda: qk_sb(wk_sb, bk_sb, xkt, kh_t, 1, 1)],
                (2, 6): [lambda: qk_sb(wq_sb, bq_sb, xqt, qh_t, 1, 0)],
                (3, 0): [lambda: qk_sb(wk_sb, bk_sb, xkt, kh_t, 1, 2)],
                (3, 2): [lambda: qk_sb(wk_sb, bk_sb, xkt, kh_t, 1, 3)],
                (3, 4): [lambda: qk_sb(wq_sb, bq_sb, xqt, qh_t, 1, 1)],
                (3, 6): [lambda: qk_sb(wq_sb, bq_sb, xqt, qh_t, 1, 2)],
                (4, 0): [lambda: qk_sb(wq_sb, bq_sb, xqt, qh_t, 1, 3)],
            }

            # last-2 ibs run their own PV within-ib (lag 2 chunks) so their
            # finish/outproj escapes the kernel tail; outproj(sb2) is spread
            # through seq 7 as fill work
            for jc in range(NST // 2):
                extras[(7, jc)] = [lambda et=jc: outproj_et(2, et)]

            seq = [(0, 0), (0, 1), (0, 2), (0, 3), (1, 0), (1, 1), (1, 2), (1, 3)]
            prev = None  # (pr, ib, at_list) with pending lag-1 PV
            ppv = None
            for seq_idx, (pr, ib) in enumerate(seq):
                within = seq_idx >= 6
                at_list = []
                lpv = None
                for jc in range(NST // 2):
                    at_list.append(scores_chunk(pr, ib, jc, seq_idx))
                    for fn in extras.get((seq_idx, jc), ()):
                        fn()
                    if prev is not None:
                        if jc == 0:
                            ppv = [ps.tile([128, 512], f32, tag="w1", bufs=4,
                                           name=f"pv{prev[0]}{prev[1]}_{i}")
                                   for i in range(2)]
                        pv_chunk(prev[0], ppv, prev[2][jc], jc)
                    if within and jc >= 2:
                        if jc == 2:
                            lpv = [ps.tile([128, 512], f32, tag="w1", bufs=4,
                                           name=f"pvlast{seq_idx}_{i}")
                                   for i in range(2)]
                        pv_chunk(pr, lpv, at_list[jc - 2], jc - 2)
                if prev is not None:
                    finish_ib(prev[0], prev[1], ppv,
                              with_outproj=(prev[0] == 1 and prev[1] < 2))
                if within:
                    pv_chunk(pr, lpv, at_list[6], 6)
                    pv_chunk(pr, lpv, at_list[7], 7)
                    finish_ib(pr, ib, lpv, with_outproj=(ib == 3))
                    prev = None
                else:
                    prev = (pr, ib, at_list)

    nc.compile()
    return nc


def _get_nc():
    if "nc" not in _compiled:
        _compiled["nc"] = _build()
    return _compiled["nc"]


def kernel(q, k, v, Wq, bq, Wk, bk, Wv, bv, Wo, bo):
    outp, _ = _run(q, k, v, Wq, bq, Wk, bk, Wv, bv, Wo, bo)
    return outp


def _run(q, k, v, Wq, bq, Wk, bk, Wv, bv, Wo, bo, **run_kwargs):
    from concourse.bass_utils import run_bass_kernel_spmd

    nc = _get_nc()

    q = np.asarray(q, np.float32)
    k = np.asarray(k, np.float32)
    v = np.asarray(v, np.float32)
    Wq = np.asarray(Wq, np.float32)
    Wk = np.asarray(Wk, np.float32)
    Wv = np.asarray(Wv, np.float32)
    Wo = np.asarray(Wo, np.float32)
    bq = np.asarray(bq, np.float32)
    bk = np.asarray(bk, np.float32)
    bv = np.asarray(bv, np.float32)
    bo = np.asarray(bo, np.float32)

    import ml_dtypes
    bf = ml_dtypes.bfloat16
    xqT = [np.ascontiguousarray(q[b].T).astype(bf) for b in range(B)]
    xkT = [np.ascontiguousarray(k[b].T).astype(bf) for b in range(B)]
    xvT = [np.ascontiguousarray(v[b].T).astype(bf) for b in range(B)]

    in_maps = []
    for c in range(NCORES):
        b, hg = divmod(c, HG)
        rows = slice(hg * F, (hg + 1) * F)
        in_maps.append({
            "xq": xqT[b], "xk": xkT[b], "xv": xvT[b],
            "wq": np.ascontiguousarray(Wq[rows].T).astype(bf),
            "wk": np.ascontiguousarray(Wk[rows].T).astype(bf),
            "wv": np.ascontiguousarray(Wv[rows].T).astype(bf),
            "wo": np.ascontiguousarray(Wo[:, rows].T).astype(bf),
            "bq": np.ascontiguousarray(bq[rows].reshape(2, 128).T),
            "bk": np.ascontiguousarray(bk[rows].reshape(2, 128).T),
            "bv": np.ascontiguousarray(bv[rows].reshape(1, F)),
        })

    res = run_bass_kernel_spmd(nc, in_maps, core_ids=list(range(NCORES)), **run_kwargs)

    outp = np.empty((B, S, D), np.float32)
    for b in range(B):
        acc = res.results[b * HG]["out"].astype(np.float32)
        for hg in range(1, HG):
            acc = acc + res.results[b * HG + hg]["out"].astype(np.float32)
        outp[b] = acc.T + bo[None, :]
    return outp, res
